# revision 30
# baseline (speedup 1.0000x reference)
"""GroupedQueryAttention TP kernel for 8 Trainium2 NeuronCores (v3).

Problem (hardcoded from the reference):
  B=2, S=2048, E=2048, H=32 q-heads, KV=8 kv-heads, D=128, fp32 I/O.
  y = GQA(x) with QK-RMSNorm, RoPE, causal mask, out-proj.

Sharding: data-parallel over batch (2) x tensor-parallel over heads (4).
  core c: batch b=c//4, tp-rank r=c%4 -> 8 q-heads, 2 kv-groups.
  Wq/Wk/Wv column-sharded, Wo row-sharded; partial outputs reduced
  across the 4 tp-ranks of each batch group on the host (free).

v4 changes vs v2 (562us) / v3 (622us):
  - softmax denominator OFF the PE: probs accumulated across sk-chunks
    with bf16 DVE adds (390ns each, 2x mode), then ONE ones-matmul per
    head-band (512 cycles instead of n_skc*512).  -123k PE cycles.
    (v3 put half on Pool at 1.06us/add -> probs-pool backpressure
    stalled the PE; reverted.)
  - causal boundary via affine_select on Pool as in v2, but boundary
    sk-tiles ordered FIRST in each head's chain so the select latency
    hides behind the full tiles' matmuls (v3's mask-matmul split added
    224 matmul+ldweights and made things worse; reverted).
  - norm+rope DVE chain rebuilt as WIDE bf16 tensor_tensor ops (2x
    DVE mode) instead of 4 ops x 10 heads.
  - k-side rmsnorm folded into the exp's per-partition scale AP
    (rstd_k lives in [sk,1] layout at band time) - drops one DVE op
    per k block and keeps kt unnormalized.
  - PSUM->SBUF out copies and ctx*rden mult moved to Pool (idle).
  - weight DMAs batched (4-ec groups, 700ns/issue on the pool queue
    was gating startup); wo last (not needed until band 0 out-proj).
  - band fillers popped at heads 0-3 (was 1,3,5,7) so the next band's
    chunk DVE work finishes before the out-proj window closes.
"""

import math
import sys

sys.path.insert(0, "/opt/trn_rl_repo")

import numpy as np
import ml_dtypes

import concourse.bass as bass
import concourse.tile as tile
from concourse import mybir
from concourse.bass_utils import run_bass_kernel_spmd
from concourse.vector_clock import ScopedClock


def _install_ntff_hook_shim():
    """The agent image ships antenv without axon_hooks; recreate it so
    trace=True can capture NTFF profiles through libaxon_pjrt.so."""
    import types
    import ctypes
    import contextlib

    try:
        import antenv.axon_hooks  # noqa: F401
        return
    except ImportError:
        pass

    mod = types.ModuleType("antenv.axon_hooks")

    def _make_hook(so_path="/opt/axon/libaxon_pjrt.so"):
        try:
            lib = ctypes.CDLL(so_path)
        except OSError:
            return None
        if not hasattr(lib, "axon_start_nrt_profile"):
            return None
        lib.axon_start_nrt_profile.argtypes = [
            ctypes.POINTER(ctypes.c_int64),
            ctypes.c_size_t,
        ]
        lib.axon_start_nrt_profile.restype = ctypes.c_int64
        lib.axon_stop_nrt_profile.argtypes = [ctypes.c_char_p]
        lib.axon_stop_nrt_profile.restype = ctypes.c_int64

        @contextlib.contextmanager
        def _hook(output_dir, device_ids):
            import jax

            jax.devices()
            if device_ids:
                ids = (ctypes.c_int64 * len(device_ids))(*device_ids)
                rc = lib.axon_start_nrt_profile(ids, len(device_ids))
            else:
                rc = lib.axon_start_nrt_profile(None, 0)
            if rc != 0:
                raise RuntimeError(f"axon_start_nrt_profile rc={rc}")
            try:
                yield
            finally:
                n = lib.axon_stop_nrt_profile(str(output_dir).encode())
                if n < 0:
                    raise RuntimeError(f"axon_stop_nrt_profile rc={n}")

        return _hook

    _state = {}

    def get_axon_ntff_profile_hook():
        if "h" not in _state:
            _state["h"] = _make_hook()
        return _state["h"]

    def set_axon_ntff_profile_hook(hook):
        _state["h"] = hook

    mod.get_axon_ntff_profile_hook = get_axon_ntff_profile_hook
    mod.set_axon_ntff_profile_hook = set_axon_ntff_profile_hook
    sys.modules["antenv.axon_hooks"] = mod


_install_ntff_hook_shim()


F32 = mybir.dt.float32
BF16 = mybir.dt.bfloat16
AF = mybir.ActivationFunctionType
ALU = mybir.AluOpType

B, S, E = 2, 2048, 2048
H, KV, D = 32, 8, 128
TP = 4
HPC = H // TP          # 8 q-heads per core
G = KV // TP           # 2 kv-groups per core
SC = S // 128          # 16 s-chunks
ECH = E // 128         # 16 e-chunks
DQ = HPC * D           # 1024 q-proj cols per core
DKV = G * D            # 256 k (and v) proj cols per core
EPS = 1e-6
INV_SQRT_D = 1.0 / math.sqrt(D)
HD2 = D // 2
NEG_MASK = -30000.0    # exp(score + NEG_MASK) == 0 in f32

# ---------------------------------------------------------------------------
# Compat: this container's walrus codegen rejects >1 semaphore wait per
# instruction ("Too many sync wait commands").  Split extra waits onto
# preceding same-engine InstNoOp carriers.
# ---------------------------------------------------------------------------
MAXW = 1


def _dedupe_ldweights(nc, ordered):
    """Drop InstLdweights that reload the exact weights already resident in
    the PE (same tile/offset/pattern, with only matmults in between).  The
    bass legalizer emits one load per matmult unconditionally; the emit
    code orders same-lhsT matmuls back to back so ~40% of loads are
    redundant.  Sync carried by a dropped load moves to a PE NoOp."""
    for _bb, insts in ordered.items():
        cur_sig = None
        new_list = []
        for inst in insts:
            if getattr(inst, "engine", None) == mybir.EngineType.PE:
                if isinstance(inst, mybir.InstLdweights):
                    sig = str(inst.ins[0])
                    if sig == cur_sig:
                        si = inst.sync_info
                        if si is not None and (si.on_wait or si.on_update):
                            new_list.append(
                                mybir.InstNoOp(
                                    name=nc.get_next_instruction_name(),
                                    engine=inst.engine,
                                    bass_nofuse=True,
                                    sync_info=si,
                                )
                            )
                        continue
                    cur_sig = sig
                elif not isinstance(inst, (mybir.InstMatmult, mybir.InstNoOp)):
                    cur_sig = None
            new_list.append(inst)
        insts[:] = new_list


def _split_waits_in_block_lists(nc, ordered):
    _dedupe_ldweights(nc, ordered)
    for _bb, insts in ordered.items():
        new_list = []
        for inst in insts:
            si = inst.sync_info
            if si is not None and len(si.on_wait) > MAXW:
                waits = list(si.on_wait)
                extra, keep = waits[:-MAXW], waits[-MAXW:]
                for i in range(0, len(extra), MAXW):
                    nop = mybir.InstNoOp(
                        name=nc.get_next_instruction_name(),
                        engine=inst.engine,
                        bass_nofuse=True,
                        sync_info=mybir.SyncInfo(
                            on_wait=extra[i : i + MAXW], on_update=[]
                        ),
                    )
                    new_list.append(nop)
                si.on_wait = keep
            new_list.append(inst)
        insts[:] = new_list


class CompatTileContext(tile.TileContext):
    @property
    def ordered_instructions_by_block(self):
        return self.__dict__.get("_ordered_instructions_by_block")

    @ordered_instructions_by_block.setter
    def ordered_instructions_by_block(self, value):
        if isinstance(value, dict):
            _split_waits_in_block_lists(self.nc, value)
        self.__dict__["_ordered_instructions_by_block"] = value

    def _drain_and_barrier(self, tick_clock, wait_clock):
        nc = self.nc
        probe = nc.sync.nop(nofuse=True)
        wait_clock.add_sem_waits(
            probe.ins, ScopedClock({None: tick_clock.global_clock})
        )
        si = probe.ins.sync_info
        waits = list(si.on_wait) if si is not None else []
        if len(waits) > MAXW:
            si.on_wait = waits[:MAXW]
            for i in range(MAXW, len(waits), MAXW):
                n2 = nc.sync.nop(nofuse=True)
                n2.ins.sync_info = mybir.SyncInfo(
                    on_wait=waits[i : i + MAXW], on_update=[]
                )
        nc.sync.drain()
        nc.all_engine_barrier()
        assert self.sems is not None
        popped = nc._tile_sem_poison_stack.pop()
        assert popped is self._sem_poison
        nc.clear_and_free_semaphores(list(self.sems.allocated().values()))
        nc.all_engine_barrier()


# ---------------------------------------------------------------------------
# Kernel builder
# ---------------------------------------------------------------------------


def build_kernel():
    nc = bass.Bass(
        "TRN2", target_bir_lowering=False, debug=False, num_devices=8
    )

    # x^T pre-tiled on host: row (sc*128+p) col (ec*128+j) = x[sc*128+j, ec*128+p]
    xt_d = nc.declare_dram_parameter("xt_d", [S, E], BF16, isOutput=False)
    # weights pre-tiled on host: wq_t[p, ec*DQ + c] = Wq[ec*128+p, c], etc.
    wq = nc.declare_dram_parameter("wq", [128, ECH * DQ], BF16, isOutput=False)
    wkv = nc.declare_dram_parameter("wkv", [128, ECH * 2 * DKV], BF16, isOutput=False)
    wo = nc.declare_dram_parameter("wo", [128, HPC * E], BF16, isOutput=False)
    bq_d = nc.declare_dram_parameter("bq", [1, DQ], F32, isOutput=False)
    bkv_d = nc.declare_dram_parameter("bkv", [1, 2 * DKV], F32, isOutput=False)
    # packed tables: [cosq | sinq' | cosk | sink'] premultiplied on host
    cs_d = nc.declare_dram_parameter("cs", [S, 4 * D], BF16, isOutput=False)
    out_d = nc.declare_dram_parameter("out", [S, E], BF16, isOutput=True)

    with CompatTileContext(nc) as tc:
        _emit(nc, tc, xt_d, wq, wkv, wo, bq_d, bkv_d, cs_d, out_d)
    return nc


def _emit(nc, tc, xt_d, wq, wkv, wo, bq_d, bkv_d, cs_d, out_d):
    from contextlib import ExitStack

    ctx = ExitStack()
    with ctx:
        # ---- persistent tensors -------------------------------------------
        persist = ctx.enter_context(tc.tile_pool(name="persist", bufs=1))
        qt_all = persist.tile([128, HPC, S], BF16, tag="qt_all")    # Q^T per head [d, s]
        kt_all = persist.tile([128, G, S], BF16, tag="kt_all")      # K^T per group [d, s]
        v_all = persist.tile([128, G, SC, D], BF16, tag="v_all")    # V per group [s, d] chunks
        wq_sb = persist.tile([128, ECH, DQ], BF16, tag="wq_sb")
        wkv_sb = persist.tile([128, ECH, 2 * DKV], BF16, tag="wkv_sb")
        wo_sb = persist.tile([128, HPC, E], BF16, tag="wo_sb")
        bq_bc = persist.tile([128, DQ], F32, tag="bq_bc")
        bkv_bc = persist.tile([128, 2 * DKV], F32, tag="bkv_bc")
        ones_bf = persist.tile([128, 128], BF16, tag="ones_bf")
        # per-chunk rstd for q (rope mult) and k (exp scale at band time)
        rstd_all = persist.tile([128, SC, HPC + G], F32, tag="rstd_all")
        eps_t = persist.tile([128, 1], F32, tag="eps_t")
        nc.vector.memset(eps_t[:, :], EPS)
        nc.vector.memset(ones_bf[:, :], 1.0)

        # one-time loads (gpsimd queues, off the per-chunk SP path).
        # Batched in 4-ec groups: per-DMA issue on the pool sequencer is
        # ~700ns, 35 separate issues was gating the first Q projection.
        # wo is NOT needed until band 0's out-projection -> load it last.
        # first ec slice alone (fast start for chunk 0's first matmuls),
        # then larger groups
        wq_groups = [(0, 1), (1, 4), (4, 8), (8, 12), (12, 16)]
        for e0, e1 in wq_groups:
            nc.gpsimd.dma_start(out=wq_sb[:, e0:e1, :],
                                in_=wq[:, e0 * DQ : e1 * DQ])
            nc.gpsimd.dma_start(out=wkv_sb[:, e0:e1, :],
                                in_=wkv[:, e0 * 2 * DKV : e1 * 2 * DKV])
            if e1 == 1:
                nc.gpsimd.dma_start(
                    out=bq_bc[:, :], in_=bq_d[:, :].to_broadcast((128, DQ)))
                nc.gpsimd.dma_start(
                    out=bkv_bc[:, :],
                    in_=bkv_d[:, :].to_broadcast((128, 2 * DKV)))
        nc.gpsimd.dma_start(out=wo_sb[:, :, :], in_=wo[:, :])

        # ---- pools --------------------------------------------------------
        xt_pool = ctx.enter_context(tc.tile_pool(name="xt", bufs=3))
        cs_pool = ctx.enter_context(tc.tile_pool(name="cs", bufs=3))
        qsb_pool = ctx.enter_context(tc.tile_pool(name="qsb", bufs=2))
        rope_pool = ctx.enter_context(tc.tile_pool(name="rope", bufs=2))
        tmp_pool = ctx.enter_context(tc.tile_pool(name="tmpA", bufs=2))
        stat_pool = ctx.enter_context(tc.tile_pool(name="stat", bufs=4))

        psA = ctx.enter_context(tc.tile_pool(name="psA", bufs=3, space="PSUM"))
        ps_pool = ctx.enter_context(tc.tile_pool(name="ps_s", bufs=2, space="PSUM"))
        pden_pool = ctx.enter_context(tc.tile_pool(name="ps_den", bufs=1, space="PSUM"))
        pctx_pool = ctx.enter_context(tc.tile_pool(name="ps_ctx", bufs=2, space="PSUM"))

        probs_pool = ctx.enter_context(tc.tile_pool(name="probs", bufs=14))
        acc_pool = ctx.enter_context(tc.tile_pool(name="acc", bufs=4))
        den_pool = ctx.enter_context(tc.tile_pool(name="den", bufs=3))
        ctxt_pool = ctx.enter_context(tc.tile_pool(name="ctxt", bufs=1))
        osb_pool = ctx.enter_context(tc.tile_pool(name="osb", bufs=3))

        def do_chunk_a(sc):
            st = {}
            s0 = st["s0"] = sc * 128
            xt = xt_pool.tile([128, E], BF16, tag="xt")
            nc.sync.dma_start(out=xt[:, :], in_=xt_d[s0 : s0 + 128, :])
            cs_sc = cs_pool.tile([128, 4 * D], BF16, tag="cs_sc")
            nc.sync.dma_start(out=cs_sc[:, :], in_=cs_d[s0 : s0 + 128, :])
            st["cs"] = cs_sc

            # --- Q + K|V projections: three interleaved PSUM chains so the
            # three matmuls per ec share one stationary xt tile (one
            # LDWEIGHTS instead of three with ldw-opt) ---
            q_sc = qsb_pool.tile([128, DQ], BF16, tag="q_sc")
            pq0 = psA.tile([128, 512], F32, tag="pA")
            pq1 = psA.tile([128, 512], F32, tag="pA")
            pkv = psA.tile([128, 512], F32, tag="pA")
            for ec in range(ECH):
                xt_sl = xt[:, ec * 128 : (ec + 1) * 128]
                nc.tensor.matmul(
                    pq0[:, :], lhsT=xt_sl, rhs=wq_sb[:, ec, 0:512],
                    start=(ec == 0), stop=(ec == ECH - 1),
                )
                nc.tensor.matmul(
                    pq1[:, :], lhsT=xt_sl, rhs=wq_sb[:, ec, 512:1024],
                    start=(ec == 0), stop=(ec == ECH - 1),
                )
                nc.tensor.matmul(
                    pkv[:, :], lhsT=xt_sl, rhs=wkv_sb[:, ec, :],
                    start=(ec == 0), stop=(ec == ECH - 1),
                )
            for hf, pq in ((0, pq0), (1, pq1)):
                nc.vector.scalar_tensor_tensor(
                    out=q_sc[:, hf * 512 : (hf + 1) * 512], in0=pq[:, :],
                    scalar=1.0, in1=bq_bc[:, hf * 512 : (hf + 1) * 512],
                    op0=ALU.mult, op1=ALU.add,
                )
            k_sc = qsb_pool.tile([128, DKV], BF16, tag="k_sc")
            nc.vector.scalar_tensor_tensor(
                out=k_sc[:, :], in0=pkv[:, 0:DKV], scalar=1.0,
                in1=bkv_bc[:, 0:DKV], op0=ALU.mult, op1=ALU.add,
            )
            for g in range(G):
                nc.vector.scalar_tensor_tensor(
                    out=v_all[:, g, sc, :], in0=pkv[:, DKV + g * D : DKV + (g + 1) * D],
                    scalar=1.0, in1=bkv_bc[:, DKV + g * D : DKV + (g + 1) * D],
                    op0=ALU.mult, op1=ALU.add,
                )
            st["q_sc"], st["k_sc"] = q_sc, k_sc
            return st

        def do_chunk_b(st):
            s0, cs_sc = st["s0"], st["cs"]
            sc = s0 // 128
            q_sc, k_sc = st["q_sc"], st["k_sc"]
            cosq, sinq = cs_sc[:, 0:D], cs_sc[:, D : 2 * D]
            cosk, sink = cs_sc[:, 2 * D : 3 * D], cs_sc[:, 3 * D : 4 * D]

            # batched RMS stats: one Square+accum per head on ACT
            ssum = stat_pool.tile([128, HPC + G], F32, tag="ssum")
            sq2 = tmp_pool.tile([128, D], BF16, tag="sq2")
            for hh in range(HPC):
                nc.scalar.activation(
                    out=sq2[:, :], in_=q_sc[:, hh * D : (hh + 1) * D],
                    func=AF.Square, accum_out=ssum[:, hh : hh + 1],
                )
            for g in range(G):
                nc.scalar.activation(
                    out=sq2[:, :], in_=k_sc[:, g * D : (g + 1) * D],
                    func=AF.Square, accum_out=ssum[:, HPC + g : HPC + g + 1],
                )
            # rstd = exp(-0.5 * ln(ssum/D + eps)); ln/exp share one act table
            rstd_f = stat_pool.tile([128, HPC + G], F32, tag="rstd_f")
            nc.scalar.activation(
                out=rstd_f[:, :], in_=ssum[:, :], func=AF.Ln,
                bias=eps_t[:, :], scale=1.0 / D,
            )
            nc.scalar.activation(
                out=rstd_all[:, sc, :], in_=rstd_f[:, :], func=AF.Exp,
                scale=-0.5,
            )

            # --- wide bf16 rope: r = q*cos + rot(q)*sin'  (per-head rstd
            # applied to q only; k's rstd rides the exp scale at band time)
            def rope_block(src, n, cos_t, sin_t, rstd_sl, t1, u, dst):
                s3 = src[:, :].rearrange("p (h d) -> p h d", h=n)
                t3 = t1[:, :].rearrange("p (h d) -> p h d", h=n)
                u3 = u[:, :].rearrange("p (h d) -> p h d", h=n)
                d3 = dst[:, :].rearrange("p (h d) -> p h d", h=n)
                nc.vector.tensor_tensor(
                    out=t3, in0=s3,
                    in1=cos_t[:, None, :].broadcast_to((128, n, D)),
                    op=ALU.mult,
                )
                nc.vector.tensor_tensor(
                    out=u3[:, :, 0:HD2], in0=s3[:, :, HD2:D],
                    in1=sin_t[:, None, 0:HD2].broadcast_to((128, n, HD2)),
                    op=ALU.mult,
                )
                nc.vector.tensor_tensor(
                    out=u3[:, :, HD2:D], in0=s3[:, :, 0:HD2],
                    in1=sin_t[:, None, HD2:D].broadcast_to((128, n, HD2)),
                    op=ALU.mult,
                )
                if rstd_sl is None:
                    nc.vector.tensor_tensor(
                        out=dst[:, :], in0=t1[:, :], in1=u[:, :], op=ALU.add,
                    )
                else:
                    nc.vector.tensor_tensor(
                        out=t1[:, :], in0=t1[:, :], in1=u[:, :], op=ALU.add,
                    )
                    nc.vector.tensor_tensor(
                        out=d3, in0=t3,
                        in1=rstd_sl[:, :, None].broadcast_to((128, n, D)),
                        op=ALU.mult,
                    )

            q_rope = rope_pool.tile([128, DQ], BF16, tag="q_rope")
            k_rope = rope_pool.tile([128, DKV], BF16, tag="k_rope")
            t1q = tmp_pool.tile([128, DQ], BF16, tag="t1q")
            uq = tmp_pool.tile([128, DQ], BF16, tag="uq")
            t1k = tmp_pool.tile([128, DKV], BF16, tag="t1k")
            uk = tmp_pool.tile([128, DKV], BF16, tag="uk")
            rope_block(q_sc, HPC, cosq, sinq, rstd_all[:, sc, 0:HPC],
                       t1q, uq, q_rope)
            rope_block(k_sc, G, cosk, sink, rstd_all[:, sc, HPC : HPC + G],
                       t1k, uk, k_rope)

            nc.sync.dma_start_transpose(
                out=qt_all[:, :, s0 : s0 + 128], in_=q_rope[:, :]
            )
            nc.sync.dma_start_transpose(
                out=kt_all[:, :, s0 : s0 + 128], in_=k_rope[:, :]
            )

        def do_band(cq, fillers=()):
            q0 = cq * 512
            n_skc = 4 * cq + 4       # causal: sk chunks 0 .. 4cq+3
            fillers = list(fillers)
            ctxt_b = ctxt_pool.tile([128, HPC, 512], BF16, tag="ctxt_b")
            # boundary sk-tiles FIRST so their exp->affine_select latency
            # hides behind the full tiles' score/ctx matmuls
            order = list(range(4 * cq, n_skc)) + list(range(0, 4 * cq))
            for hh in range(HPC):
                # fillers at heads 1-4: head 0's exps reach the ACT queue
                # without a chunk's RMS stats in front of them
                if fillers and hh >= 1:
                    ensure_chunk(fillers.pop(0))
                g = hh // (HPC // G)
                pctx = pctx_pool.tile([128, 512], F32, tag="pctx")
                acc = acc_pool.tile([128, 512], BF16, tag="acc")
                for idx, skc in enumerate(order):
                    off = max(0, (skc - 4 * cq)) * 128
                    ps = ps_pool.tile([128, 512], F32, tag="ps")
                    nc.tensor.matmul(
                        ps[:, off:512],
                        lhsT=kt_all[:, g, skc * 128 : (skc + 1) * 128],
                        rhs=qt_all[:, hh, q0 + off : q0 + 512],
                        start=True, stop=True,
                    )
                    probs = probs_pool.tile([128, 512], BF16, tag="probs")
                    nc.scalar.activation(
                        out=probs[:, off:512], in_=ps[:, off:512], func=AF.Exp,
                    )
                    if skc >= 4 * cq:
                        # boundary tile: keep sq >= sk, i.e. j - p >= 0 on the
                        # [128,128] tile at col offset `off` (base is 0 there)
                        nc.gpsimd.affine_select(
                            out=probs[:, off : off + 128],
                            in_=probs[:, off : off + 128],
                            compare_op=ALU.is_ge, fill=0.0,
                            base=0,
                            pattern=[[1, 128]], channel_multiplier=-1,
                        )
                    if idx == 0:
                        # acc[0:128] only ever gets contributions from this
                        # tile (+ full tiles later); defer the wide copy by
                        # pair-adding tile 0 and 1 at idx==1
                        nc.vector.tensor_copy(
                            out=acc[:, 0:128], in_=probs[:, 0:128]
                        )
                        probs_first = probs
                    elif idx == 1:
                        nc.vector.tensor_tensor(
                            out=acc[:, off:512], in0=probs_first[:, off:512],
                            in1=probs[:, off:512], op=ALU.add,
                        )
                    else:
                        nc.vector.tensor_tensor(
                            out=acc[:, off:512], in0=acc[:, off:512],
                            in1=probs[:, off:512], op=ALU.add,
                        )
                    nc.tensor.matmul(
                        pctx[:, off:512], lhsT=v_all[:, g, skc, :],
                        rhs=probs[:, off:512],
                        start=(idx == 0), stop=(idx == n_skc - 1),
                        skip_group_check=(idx != 0),
                    )
                # partition-sum of the accumulated probs: ONE 512-cycle
                # ones-matmul per head-band (was n_skc of them)
                pden = pden_pool.tile([128, 512], F32, tag="pden")
                nc.tensor.matmul(
                    pden[:, :], lhsT=ones_bf[:, :], rhs=acc[:, :],
                    start=True, stop=True,
                )
                # rden = exp(-ln(den)) on ACT
                rden = den_pool.tile([128, 512], F32, tag="rden")
                nc.scalar.activation(
                    out=rden[:, :], in_=pden[:, :], func=AF.Ln,
                )
                nc.scalar.activation(
                    out=rden[:, :], in_=rden[:, :], func=AF.Exp, scale=-1.0,
                )
                nc.vector.tensor_tensor(
                    out=ctxt_b[:, hh, :], in0=pctx[:, :],
                    in1=rden[:, :], op=ALU.mult,
                )
            # out-projection for the four 128-row chunks of this sq range;
            # oc pairs run as two parallel PSUM chains sharing each ctxt
            # stationary tile (half the LDWEIGHTS with ldw-opt)
            for sq_i in range(4):
                sqc = 4 * cq + sq_i
                for op0 in (0, 2):
                    poa = psA.tile([128, 512], F32, tag="pA")
                    pob = psA.tile([128, 512], F32, tag="pA")
                    for hc in range(HPC):
                        ct_sl = ctxt_b[:, hc, sq_i * 128 : (sq_i + 1) * 128]
                        nc.tensor.matmul(
                            poa[:, :], lhsT=ct_sl,
                            rhs=wo_sb[:, hc, op0 * 512 : (op0 + 1) * 512],
                            start=(hc == 0), stop=(hc == HPC - 1),
                        )
                        nc.tensor.matmul(
                            pob[:, :], lhsT=ct_sl,
                            rhs=wo_sb[:, hc, (op0 + 1) * 512 : (op0 + 2) * 512],
                            start=(hc == 0), stop=(hc == HPC - 1),
                        )
                    for oc, po in ((op0, poa), (op0 + 1, pob)):
                        osb = osb_pool.tile([128, 512], BF16, tag="osb")
                        # split the PSUM->SBUF copy burst across DVE and ACT
                        # (gpsimd cannot touch PSUM)
                        if oc % 2 == 0:
                            nc.vector.tensor_copy(out=osb[:, :], in_=po[:, :])
                        else:
                            nc.scalar.activation(out=osb[:, :], in_=po[:, :],
                                                 func=AF.Copy)
                        nc.sync.dma_start(
                            out=out_d[sqc * 128 : (sqc + 1) * 128,
                                      oc * 512 : (oc + 1) * 512],
                            in_=osb[:, :],
                        )

        # ---- interleaved schedule: chunks 4b..4b+3 then band b, with the
        # next band's chunks emitted during the band's early heads so the
        # DVE/ACT chunk work finishes before the out-proj window closes ----
        emitted = set()

        def ensure_chunk(sc):
            if sc < SC and sc not in emitted:
                emitted.add(sc)
                do_chunk_b(do_chunk_a(sc))

        for bnd in range(4):
            for sc in range(4 * bnd, 4 * bnd + 4):
                ensure_chunk(sc)
            do_band(bnd, fillers=[4 * bnd + 4 + i for i in range(4)])


_NC_CACHE = {}


def _get_nc():
    if "nc" not in _NC_CACHE:
        _NC_CACHE["nc"] = build_kernel()
    return _NC_CACHE["nc"]


def _prep_tables(cos, sin, q_scale, k_scale):
    """Pack [cosq|sinq'|cosk|sink'] with scales, rope signs and 1/sqrt(D)
    folded in.  sin'[j<64] = -sin[j]*scale[j+64]; sin'[j>=64] = sin[j]*scale[j-64]."""
    cos = cos.astype(np.float64)
    sin = sin.astype(np.float64)

    def fold(scale, isd):
        scale = scale.astype(np.float64)
        cos_t = cos * scale * isd
        sin_t = np.empty_like(sin)
        sin_t[:, :HD2] = -sin[:, :HD2] * scale[HD2:] * isd
        sin_t[:, HD2:] = sin[:, HD2:] * scale[:HD2] * isd
        return cos_t, sin_t

    cq, sq = fold(q_scale, INV_SQRT_D)
    ck, sk = fold(k_scale, 1.0)
    return np.concatenate([cq, sq, ck, sk], axis=1).astype(ml_dtypes.bfloat16)


def _shard_inputs(x, mask, cos, sin, Wq, bq, Wk, bk, Wv, bv, Wo, q_scale, k_scale):
    bf = ml_dtypes.bfloat16
    # x^T in PE-tile layout: xt_d[sc*128+p, ec*128+j] = x[sc*128+j, ec*128+p]
    xt_b = []
    for b in range(B):
        xb = np.asarray(x[b], dtype=np.float32)
        t = xb.reshape(SC, 128, ECH, 128).transpose(0, 3, 2, 1)  # [sc, p, ec, j]
        xt_b.append(np.ascontiguousarray(t.reshape(S, E)).astype(bf))
    cs = _prep_tables(cos, sin, q_scale, k_scale)
    in_maps = []
    for c in range(8):
        b, r = c // TP, c % TP
        def tile_rows(w):
            # [E_rows, C] -> [128, nch*C] with w_t[p, ec*C + c] = w[ec*128+p, c]
            nch = w.shape[0] // 128
            return np.ascontiguousarray(
                w.reshape(nch, 128, w.shape[1]).transpose(1, 0, 2).reshape(128, -1)
            ).astype(bf)

        wq_r = tile_rows(Wq[:, r * DQ : (r + 1) * DQ])
        wk_r = Wk[:, r * DKV : (r + 1) * DKV]
        wv_r = Wv[:, r * DKV : (r + 1) * DKV]
        wkv_r = tile_rows(np.concatenate([wk_r, wv_r], axis=1))
        wo_r = tile_rows(np.asarray(Wo[r * DQ : (r + 1) * DQ, :]))
        bq_r = np.ascontiguousarray(bq[r * DQ : (r + 1) * DQ]).reshape(1, DQ).astype(np.float32)
        bkv_r = np.concatenate(
            [bk[r * DKV : (r + 1) * DKV], bv[r * DKV : (r + 1) * DKV]]
        ).reshape(1, 2 * DKV).astype(np.float32)
        in_maps.append(
            {
                "xt_d": xt_b[b],
                "wq": wq_r,
                "wkv": wkv_r,
                "wo": wo_r,
                "bq": bq_r,
                "bkv": bkv_r,
                "cs": cs,
            }
        )
    return in_maps


def kernel(x, mask, cos, sin, Wq, bq, Wk, bk, Wv, bv, Wo, q_scale, k_scale,
           _trace=False, _trace_kwargs=None):
    x = np.asarray(x, dtype=np.float32)
    in_maps = _shard_inputs(
        x, mask, np.asarray(cos), np.asarray(sin),
        np.asarray(Wq), np.asarray(bq), np.asarray(Wk), np.asarray(bk),
        np.asarray(Wv), np.asarray(bv), np.asarray(Wo),
        np.asarray(q_scale), np.asarray(k_scale),
    )
    nc = _get_nc()
    res = run_bass_kernel_spmd(
        nc, in_maps, list(range(8)), trace=_trace,
        **(_trace_kwargs or {}),
    )
    out = np.zeros((B, S, E), dtype=np.float32)
    for c in range(8):
        b = c // TP
        out[b] += res.results[c]["out"].astype(np.float32)
    if _trace:
        kernel._last_result = res
    return out
